# revision 28
# baseline (speedup 1.0000x reference)
"""GCN2 (GCNII) message-passing kernel for 8 Trainium2 NeuronCores.

Strategy (1D node partition, hint-compliant):
  - Nodes sharded 8 ways by id; each core owns 12500 nodes (padded to 12544).
  - Layer weights replicated; per-layer halo exchange realized as 4 chunked
    AllGathers of each core's row shard into a replicated bf16 DRAM table.
  - Symmetric normalization folded into the data path:
      table rows store r = dinv * h  (bf16); per-dst scale v09 = 0.9*dinv[dst]
      applied AFTER aggregation (per-partition scalar in rows-major layout).
  - Self-loops never gathered: their contribution is v09[d] * r[d] using the
    resident local rows, fused into the z computation.
  - Aggregation (segment_sum over dst-sorted edges): dma_gather of 256B bf16
    rows by src, then per-128-edge-chunk one-hot matmuls on the TensorEngine
    accumulating into PSUM (S[dst,feat] += onehot^T @ G).
    One-hot indicators are PURE 0/1, layer-invariant, precomputed on host and
    streamed from DRAM (no per-chunk VectorEngine builds).
  - Edges grouped by (dst tile of 128, src bank of 32768 rows) because
    dma_gather indices are int16; per-core padding is trailing -1 indices per
    (tile,bank) gather call, which the gather ucode drops (no descriptors, no
    HBM fetch for padding).
  - Transform matmuls (z @ W) and input/output layers kept in fp32 (cheap,
    preserves accuracy); only the gather/aggregate path is bf16.
"""

import math
import os
import sys

import numpy as np

for _p in ("/opt/trn_rl_repo",):
    if _p not in sys.path and os.path.isdir(_p):
        sys.path.insert(0, _p)

import ml_dtypes

import concourse.bacc as bacc
import concourse.mybir as mybir
import concourse.tile as tile
from concourse.bass_utils import run_bass_kernel_spmd

# ---------------- problem constants (hardcoded per contract) ----------------
N = 100_000
E = 1_600_000
IN_C = 500
HID = 128
OUT_C = 64
L = 8
ALPHA = 0.1
THETA = 0.5

NCORES = 8
NOWN = N // NCORES          # 12500 real nodes per core
NLOC = 12544                # padded to 98 * 128
NT = NLOC // 128            # 98 dst tiles per core
TGS = 4                     # dst tiles per load group
KIN = 512                   # padded input channels
BANK = 32768                # int16-addressable rows per gather bank
AGC = 4096                  # shard rows per chunked AllGather (AG 0..2)
TROWS = 3 * BANK + NCORES * (NLOC - 3 * AGC)   # 100352 table rows

F32 = mybir.dt.float32
BF16 = mybir.dt.bfloat16
I16 = mybir.dt.int16
BETAS = [float(np.log(THETA / (l + 1) + 1.0)) for l in range(L)]

_cache = {}

LAST_PERF = {}


def _row_of_node(n):
    """Table row of global node id(s) n (vectorized)."""
    c = n // NOWN
    i = n - c * NOWN
    q = np.minimum(i // AGC, 3)
    tail = NLOC - 3 * AGC  # 256
    return np.where(q < 3, q * BANK + c * AGC + (i - q * AGC),
                    3 * BANK + c * tail + (i - 3 * AGC))


def _preprocess(edge_index):
    """All graph-structure preprocessing on host (numpy)."""
    e0 = edge_index[0].astype(np.int64)
    e1 = edge_index[1].astype(np.int64)
    # degrees INCLUDE self-loops (gcn_norm adds them), but self-loop edges are
    # folded on-chip and never gathered.
    deg = np.bincount(e1, minlength=N).astype(np.float64) + 1.0
    dinv = (1.0 / np.sqrt(deg)).astype(np.float32)

    src, dst = e0, e1
    owner = dst // NOWN
    ldst = dst - owner * NOWN
    t_arr = ldst // 128
    dl_arr = ldst % 128
    row = _row_of_node(src)
    b_arr = row // BANK
    bidx = (row % BANK).astype(np.int16)

    # group id and stable sort: (core, tile-group, bank, tile-in-group).
    # Bank-major within a tile group so each (tg, bank) is one contiguous
    # chunk range = one dma_gather call.
    NTG = (NT + TGS - 1) // TGS
    tg_arr = t_arr // TGS
    tl_arr = t_arr % TGS
    G = (((owner * NTG + tg_arr) * 4 + b_arr) * TGS + tl_arr).astype(np.int64)
    order = np.argsort(G, kind="stable")
    Gs = G[order]
    counts_g = np.bincount(Gs, minlength=NCORES * NTG * 4 * TGS)
    counts_g = counts_g.reshape(NCORES, NTG, 4, TGS)
    Cg = np.ceil(counts_g / 128).astype(np.int64).max(axis=0)  # [NTG, 4, TGS]

    # chunk offsets in (tg, bank, tile) order
    flat = Cg.reshape(-1)
    g_ch0 = np.zeros(len(flat) + 1, np.int64)
    np.cumsum(flat, out=g_ch0[1:])
    NCHUNK = int(g_ch0[-1])
    NSLOT = NCHUNK * 128
    ch_of = g_ch0[:-1].reshape(NTG, 4, TGS)   # first chunk of (tg, b, tl)
    slot0 = ch_of * 128

    # per-tile scattered chunk lists and per-(tg,bank) contiguous ranges
    tile_chunks = [
        [int(ch_of[t // TGS, b, t % TGS]) + k
         for b in range(4) for k in range(int(Cg[t // TGS, b, t % TGS]))]
        for t in range(NT)
    ]
    bank_rng = [
        [(int(ch_of[g, b, 0]),
          int(ch_of[g, b, 0]) + int(Cg[g, b, :].sum())) for b in range(4)]
        for g in range(NTG)
    ]

    # fill per-core flat arrays (vectorized scatter)
    gstart = np.zeros(NCORES * NTG * 4 * TGS + 1, np.int64)
    np.cumsum(counts_g.reshape(-1), out=gstart[1:])
    rank = np.arange(len(Gs)) - gstart[Gs]
    core_of = Gs // (NTG * 4 * TGS)
    tb = Gs % (NTG * 4 * TGS)
    dest = core_of * NSLOT + slot0.reshape(-1)[tb] + rank

    # padding gathers row 0 (indicator columns for padded slots are zero);
    # -1 trailing-trim is NOT used: it wedges the device, and with bank-major
    # batched gather calls padding is interior anyway.
    idx_flat = np.zeros(NCORES * NSLOT, np.int16)
    idx_flat[dest] = bidx[order]
    ind_flat = np.zeros((NCORES * NSLOT, 128), ml_dtypes.bfloat16)
    ind_flat[dest, dl_arr[order]] = 1.0

    idx_flat = idx_flat.reshape(NCORES, NSLOT)
    # idx device layout: [c, 128, NSLOT//16] (16-wrapped, replicated x8)
    idx_w = np.tile(
        idx_flat.reshape(NCORES, NSLOT // 16, 16).transpose(0, 2, 1), (1, 8, 1)
    ).copy()
    # ind device layout: [c, 128(slot%128), NCHUNK, 128(dl)]
    ind_w = np.ascontiguousarray(
        ind_flat.reshape(NCORES, NCHUNK, 128, 128).transpose(0, 2, 1, 3)
    ).reshape(NCORES, 128, NCHUNK * 128)

    dinv_loc = np.zeros((NCORES, NLOC), np.float32)
    dinv_loc[:, :NOWN] = dinv.reshape(NCORES, NOWN)
    dinv_col = np.ascontiguousarray(
        dinv_loc.reshape(NCORES, NT, 128).transpose(0, 2, 1))
    v09_col = (0.9 * dinv_col).astype(np.float32)

    return dict(tile_chunks=tile_chunks, bank_rng=bank_rng,
                NCHUNK=NCHUNK, NSLOT=NSLOT,
                idx_w=idx_w, ind_w=ind_w,
                dinv_col=dinv_col, v09_col=v09_col, counts=counts_g)


def _build_program(pre, n_layers=L, max_tg=None):
    nc = bacc.Bacc("TRN2", target_bir_lowering=False, debug=False,
                   num_devices=NCORES, num_swdge_queues=4)
    tile_chunks = pre["tile_chunks"]
    bank_rng = pre["bank_rng"]
    NCHUNK, NSLOT = pre["NCHUNK"], pre["NSLOT"]

    # ---- external inputs ----
    xT_in = nc.dram_tensor("xT", [128, KIN // 128, NLOC], F32, kind="ExternalInput")
    win_in = nc.dram_tensor("win", [128, KIN // 128, HID], F32, kind="ExternalInput")
    bin_in = nc.dram_tensor("bin", [128, 1], F32, kind="ExternalInput")
    wc_in = nc.dram_tensor("wc", [128, L, HID], F32, kind="ExternalInput")
    wout_in = nc.dram_tensor("wout", [128, OUT_C], F32, kind="ExternalInput")
    bout_in = nc.dram_tensor("bout", [128, OUT_C], F32, kind="ExternalInput")
    identf_in = nc.dram_tensor("identf", [128, 128], F32, kind="ExternalInput")
    identb_in = nc.dram_tensor("identb", [128, 128], BF16, kind="ExternalInput")
    dinv_in = nc.dram_tensor("dinvc", [128, NT], F32, kind="ExternalInput")
    v09_in = nc.dram_tensor("v09c", [128, NT], F32, kind="ExternalInput")
    idx_in = nc.dram_tensor("idx", [128, NSLOT // 16], I16, kind="ExternalInput")
    ind_in = nc.dram_tensor("ind", [128, NSLOT], BF16, kind="ExternalInput")
    out_ext = nc.dram_tensor("out", [NOWN, OUT_C], F32, kind="ExternalOutput")

    rg = [list(range(NCORES))]

    tg_list = [list(range(g, min(g + TGS, NT))) for g in range(0, NT, TGS)]
    if max_tg is not None:
        tg_list = tg_list[:max_tg]
    tg_c0 = [bank_rng[gi][0][0] for gi in range(len(tg_list))]
    tg_c1 = [bank_rng[gi][3][1] for gi in range(len(tg_list))]
    max_nch = max(c1 - c0 for c0, c1 in zip(tg_c0, tg_c1))

    with tile.TileContext(nc, num_cores=NCORES) as tc:
        with (
            tc.tile_pool(name="const", bufs=1) as cpool,
            tc.tile_pool(name="dram", bufs=1, space="DRAM") as dram,
            tc.tile_pool(name="work", bufs=1) as wp,
            tc.tile_pool(name="psum", bufs=1, space="PSUM") as pp,
        ):
            # ---- resident constants ----
            win_sb = cpool.tile([128, KIN // 128, HID], F32)
            bin_sb = cpool.tile([128, 1], F32)
            wc_sb = cpool.tile([128, L, HID], F32)
            wout_sb = cpool.tile([128, OUT_C], F32)
            bout_sb = cpool.tile([128, OUT_C], F32)
            identf_sb = cpool.tile([128, 128], F32)
            identb_sb = cpool.tile([128, 128], BF16)
            dinv_sb = cpool.tile([128, NT], F32)
            v09_sb = cpool.tile([128, NT], F32)
            h0s = cpool.tile([128, NT, HID], BF16)     # 0.1*h0, rows-major
            rows_buf = cpool.tile([128, NT, HID], BF16)  # dinv*h rows, resident
            for sb_t, ext in ((win_sb, win_in), (bin_sb, bin_in), (wc_sb, wc_in),
                              (wout_sb, wout_in), (bout_sb, bout_in),
                              (identf_sb, identf_in), (identb_sb, identb_in),
                              (dinv_sb, dinv_in), (v09_sb, v09_in)):
                nc.sync.dma_start(sb_t[:], ext[:])

            # gather destination buffers: explicit triple buffer, zeroed once
            # (padded slots gather row 0; indicator columns for them are zero)
            gbufs = [cpool.tile([128, max_nch, HID], BF16, name=f"gb{i}")
                     for i in range(3)]
            for g in gbufs:
                nc.vector.memset(g[:], 0.0)

            # one Shared table tensor per (layer, bank)
            NBROWS = [BANK, BANK, BANK, TROWS - 3 * BANK]
            tables = [
                [dram.tile([NBROWS[b], HID], BF16, addr_space="Shared",
                           name=f"table{r}_{b}") for b in range(4)]
                for r in range(L)
            ]
            shards = [
                dram.tile([NLOC, HID], BF16, name=f"shard{i}") for i in range(2)
            ]

            AG_BOUNDS = [(0, AGC), (AGC, 2 * AGC), (2 * AGC, 3 * AGC),
                         (3 * AGC, NLOC)]
            AG_LAG = 10   # tiles of slack before firing, so the Pool queue
                          # does not stall waiting for the shard writes

            def ag_round(shard, tb, r):
                """AllGather round r: shard rows -> table bank r.

                Fired mid-layer (inputs long written) so the transfer overlaps
                the remainder of the layer instead of serializing at the
                boundary behind the whole layer's dma_gathers."""
                r0, r1 = AG_BOUNDS[r]
                nc.gpsimd.collective_compute(
                    "AllGather", mybir.AluOpType.bypass, replica_groups=rg,
                    ins=[shard[r0:r1, :].opt()],
                    outs=[tb[r][:, :].opt()],
                )

            def fire_due_ags(shard, tb, done_tile, state, lag=AG_LAG):
                while state["next"] < 4 and \
                        done_tile + 1 >= AG_BOUNDS[state["next"]][1] // 128 + lag:
                    ag_round(shard, tb, state["next"])
                    state["next"] += 1

            def fire_rest_ags(shard, tb, state):
                while state["next"] < 4:
                    ag_round(shard, tb, state["next"])
                    state["next"] += 1

            # ================= input layer: h0 = relu(x @ W_in + b_in) ======
            shard0 = shards[0]
            ag_state = {"next": 0}
            for g0 in range(0, NLOC, 512):
                w = min(512, NLOC - g0)
                psin = pp.tile([128, 512], F32, tag="pi", bufs=1)
                for k in range(KIN // 128):
                    xt = wp.tile([128, 512], F32, tag="xt", bufs=3)
                    nc.sync.dma_start(xt[:, :w], xT_in[:, k, g0:g0 + w])
                    nc.tensor.matmul(psin[:, :w], win_sb[:, k, :], xt[:, :w],
                                     start=(k == 0), stop=(k == KIN // 128 - 1))
                h0T = wp.tile([128, 512], F32, tag="h0T", bufs=2)
                nc.scalar.activation(h0T[:, :w], psin[:, :w],
                                     mybir.ActivationFunctionType.Relu,
                                     bias=bin_sb[:, 0:1])
                for j in range(w // 128):
                    t = g0 // 128 + j
                    tp = pp.tile([128, 128], F32, tag="zt", bufs=2)
                    nc.tensor.transpose(tp[:], h0T[:, j * 128:(j + 1) * 128],
                                        identf_sb[:])
                    nc.vector.tensor_scalar(h0s[:, t, :], tp[:], ALPHA, None,
                                            mybir.AluOpType.mult)
                    nc.scalar.activation(rows_buf[:, t, :], tp[:],
                                         mybir.ActivationFunctionType.Copy,
                                         scale=dinv_sb[:, t:t + 1])
                    nc.scalar.dma_start(shard0[t * 128:(t + 1) * 128, :],
                                        rows_buf[:, t, :])
                    fire_due_ags(shard0, tables[0], t, ag_state, lag=2)
            fire_rest_ags(shard0, tables[0], ag_state)

            # ========================= L layers =============================
            for l in range(n_layers):
                t_in = tables[l]
                shard = shards[(l + 1) % 2]
                last = l == L - 1
                ag_state = {"next": 0}
                skips = os.environ.get("GCN_SKIP", "").split(",")
                for gi, tiles in enumerate(tg_list):
                    c0 = tg_c0[gi]
                    c1 = tg_c1[gi]
                    nch = c1 - c0
                    idx_t = wp.tile([128, nch * 8], I16, tag="idxs", bufs=3)
                    nc.gpsimd.dma_start(idx_t[:], idx_in[:, c0 * 8:c1 * 8])
                    ind_t = wp.tile([128, nch * 128], BF16, tag="ind", bufs=3)
                    nc.sync.dma_start(ind_t[:], ind_in[:, c0 * 128:c1 * 128])
                    gbuf = gbufs[gi % 3]
                    # one gather per (tile-group, bank): bank-major slot layout
                    # makes each bank's chunks contiguous across the group
                    for b in range(4):
                        if "gather" in skips:
                            continue
                        b0, b1 = bank_rng[gi][b]
                        cnt = b1 - b0
                        if cnt == 0:
                            continue
                        rel = b0 - c0
                        nc.gpsimd.dma_gather(
                            gbuf[:, rel:rel + cnt, :],
                            t_in[b][:, :],
                            idx_t[:, rel * 8:(rel + cnt) * 8],
                            cnt * 128, cnt * 128, HID,
                            single_packet=False,
                            queue_num=(gi + b) % 4,
                        )

                    ps_g = pp.tile([128, 512], F32, tag="ps", bufs=2)
                    for ti, t in enumerate(tiles):
                        chunks = [p - c0 for p in tile_chunks[t]]
                        if "mm" in skips:
                            chunks = []
                        ps = ps_g[:, ti * 128:(ti + 1) * 128]
                        for k, p in enumerate(chunks):
                            nc.tensor.matmul(ps, ind_t[:, p * 128:(p + 1) * 128],
                                             gbuf[:, p, :],
                                             start=(k == 0),
                                             stop=(k == len(chunks) - 1))
                    for ti, t in enumerate(tiles):
                        ps = ps_g[:, ti * 128:(ti + 1) * 128]
                        # z = v09*(S + r_prev) + 0.1*h0   (self-loop folded)
                        t1 = wp.tile([128, 128], F32, tag="t1", bufs=2)
                        nc.vector.tensor_tensor(t1[:], ps, rows_buf[:, t, :],
                                                mybir.AluOpType.add)
                        z = wp.tile([128, 128], F32, tag="z", bufs=2)
                        nc.vector.scalar_tensor_tensor(
                            z[:], t1[:], v09_sb[:, t:t + 1], h0s[:, t, :],
                            mybir.AluOpType.mult, mybir.AluOpType.add)
                        # zT for the transform matmul
                        ztp = pp.tile([128, 128], F32, tag="zt", bufs=2)
                        nc.tensor.transpose(ztp[:], z[:], identf_sb[:])
                        zt = wp.tile([128, 128], F32, tag="ztc", bufs=2)
                        nc.scalar.activation(zt[:], ztp[:],
                                             mybir.ActivationFunctionType.Copy)
                        psh = pp.tile([128, 128], F32, tag="ph", bufs=2)
                        nc.tensor.matmul(psh[:], zt[:], wc_sb[:, l, :],
                                         start=True, stop=True)
                        pre = wp.tile([128, 128], F32, tag="pre", bufs=2)
                        nc.vector.scalar_tensor_tensor(
                            pre[:], z[:], 1.0 - BETAS[l], psh[:],
                            mybir.AluOpType.mult, mybir.AluOpType.add)
                        if not last:
                            nc.scalar.activation(
                                rows_buf[:, t, :], pre[:],
                                mybir.ActivationFunctionType.Relu,
                                scale=dinv_sb[:, t:t + 1])
                            nc.scalar.dma_start(
                                shard[t * 128:(t + 1) * 128, :],
                                rows_buf[:, t, :])
                            fire_due_ags(shard, tables[l + 1], t, ag_state)
                        else:
                            h = wp.tile([128, 128], F32, tag="h", bufs=2)
                            nc.scalar.activation(
                                h[:], pre[:],
                                mybir.ActivationFunctionType.Relu)
                            htp = pp.tile([128, 128], F32, tag="zt", bufs=2)
                            nc.tensor.transpose(htp[:], h[:], identf_sb[:])
                            ht = wp.tile([128, 128], F32, tag="ztc", bufs=2)
                            nc.scalar.activation(
                                ht[:], htp[:],
                                mybir.ActivationFunctionType.Copy)
                            pso = pp.tile([128, OUT_C], F32, tag="po", bufs=1)
                            nc.tensor.matmul(pso[:], ht[:], wout_sb[:],
                                             start=True, stop=True)
                            ob = wp.tile([128, OUT_C], F32, tag="ob", bufs=3)
                            nc.vector.tensor_tensor(ob[:], pso[:], bout_sb[:],
                                                    mybir.AluOpType.add)
                            r0 = t * 128
                            r1 = min(r0 + 128, NOWN)
                            if r1 > r0:
                                nc.scalar.dma_start(out_ext[r0:r1, :],
                                                    ob[:r1 - r0, :])
                    # end tiles
                if not last:
                    fire_rest_ags(shard, tables[l + 1], ag_state)

    nc.compile()
    return nc


def _host_inputs(inputs, pre):
    x = np.asarray(inputs["x"], np.float32)
    W_in = np.asarray(inputs["W_in"], np.float32)
    b_in = np.asarray(inputs["b_in"], np.float32)
    W_conv = np.asarray(inputs["W_conv"], np.float32)
    W_out = np.asarray(inputs["W_out"], np.float32)
    b_out = np.asarray(inputs["b_out"], np.float32)
    betas = np.array(BETAS, np.float32)

    win_w = np.zeros((128, KIN // 128, HID), np.float32)
    for k in range(KIN // 128):
        rows = W_in[k * 128:min((k + 1) * 128, IN_C)]
        win_w[:rows.shape[0], k, :] = rows
    wc_w = (W_conv * betas[:, None, None]).transpose(1, 0, 2).copy()  # [128,L,128]
    identf_w = np.eye(128, dtype=np.float32)
    identb_w = np.eye(128, dtype=ml_dtypes.bfloat16)
    bout_w = np.tile(b_out[None, :], (128, 1)).astype(np.float32)
    bin_w = b_in.reshape(128, 1).astype(np.float32)

    xT_w = np.zeros((NCORES, 128, KIN // 128, NLOC), np.float32)
    xr = x.reshape(NCORES, NOWN, IN_C)
    for k in range(KIN // 128):
        c0, c1 = k * 128, min((k + 1) * 128, IN_C)
        xT_w[:, :c1 - c0, k, :NOWN] = xr[:, :, c0:c1].transpose(0, 2, 1)

    maps = []
    for c in range(NCORES):
        maps.append({
            "xT": xT_w[c], "win": win_w, "bin": bin_w, "wc": wc_w,
            "wout": W_out, "bout": bout_w,
            "identf": identf_w, "identb": identb_w,
            "dinvc": pre["dinv_col"][c], "v09c": pre["v09_col"][c],
            "idx": pre["idx_w"][c], "ind": pre["ind_w"][c],
        })
    return maps


def kernel(**inputs):
    edge_index = np.asarray(inputs["edge_index"])
    key = hash(edge_index.tobytes())
    if key not in _cache:
        pre = _preprocess(edge_index)
        n_layers = int(os.environ.get("GCN_NL", L))
        max_tg = os.environ.get("GCN_MAXTG")
        nc = _build_program(pre, n_layers,
                            int(max_tg) if max_tg else None)
        _cache.clear()
        _cache[key] = (pre, nc)
    pre, nc = _cache[key]

    in_maps = _host_inputs(inputs, pre)
    trace = bool(os.environ.get("GCN_TRACE"))
    res = run_bass_kernel_spmd(nc, in_maps, core_ids=list(range(NCORES)),
                               trace=trace)
    LAST_PERF["exec_time_ns"] = res.exec_time_ns
    LAST_PERF["mean_exec_time_ns"] = res.mean_exec_time_ns
    LAST_PERF["trace"] = (res.instructions_and_trace or (None, None))[1]
    out = np.concatenate([res.results[c]["out"] for c in range(NCORES)], axis=0)
    return out.astype(np.float32)


# revision 30
# speedup vs baseline: 1.0387x; 1.0387x over previous
"""GCN2 (GCNII) message-passing kernel for 8 Trainium2 NeuronCores.

Strategy (1D node partition, hint-compliant):
  - Nodes sharded 8 ways by id; each core owns 12500 nodes (padded to 12544).
  - Layer weights replicated; per-layer halo exchange realized as 4 chunked
    AllGathers of each core's row shard into a replicated bf16 DRAM table.
  - Symmetric normalization folded into the data path:
      table rows store r = dinv * h  (bf16); per-dst scale v09 = 0.9*dinv[dst]
      applied AFTER aggregation (per-partition scalar in rows-major layout).
  - Self-loops never gathered: their contribution is v09[d] * r[d] using the
    resident local rows, fused into the z computation.
  - Aggregation (segment_sum over dst-sorted edges): dma_gather of 256B bf16
    rows by src, then per-128-edge-chunk one-hot matmuls on the TensorEngine
    accumulating into PSUM (S[dst,feat] += onehot^T @ G).
    One-hot indicators are PURE 0/1, layer-invariant, precomputed on host and
    streamed from DRAM (no per-chunk VectorEngine builds).
  - Edges grouped by (dst tile of 128, src bank of 32768 rows) because
    dma_gather indices are int16; per-core padding is trailing -1 indices per
    (tile,bank) gather call, which the gather ucode drops (no descriptors, no
    HBM fetch for padding).
  - Transform matmuls (z @ W) and input/output layers kept in fp32 (cheap,
    preserves accuracy); only the gather/aggregate path is bf16.
"""

import math
import os
import sys

import numpy as np

for _p in ("/opt/trn_rl_repo",):
    if _p not in sys.path and os.path.isdir(_p):
        sys.path.insert(0, _p)

import ml_dtypes

import concourse.bacc as bacc
import concourse.mybir as mybir
import concourse.tile as tile
from concourse.bass_utils import run_bass_kernel_spmd

# ---------------- problem constants (hardcoded per contract) ----------------
N = 100_000
E = 1_600_000
IN_C = 500
HID = 128
OUT_C = 64
L = 8
ALPHA = 0.1
THETA = 0.5

NCORES = 8
NOWN = N // NCORES          # 12500 real nodes per core
NLOC = 12544                # padded to 98 * 128
NT = NLOC // 128            # 98 dst tiles per core
TGS = 4                     # dst tiles per load group
KIN = 512                   # padded input channels
BANK = 32768                # int16-addressable rows per gather bank
AGC = 4096                  # shard rows per chunked AllGather (AG 0..2)
TROWS = 3 * BANK + NCORES * (NLOC - 3 * AGC)   # 100352 table rows

F32 = mybir.dt.float32
BF16 = mybir.dt.bfloat16
FP8 = mybir.dt.float8e4
I16 = mybir.dt.int16
BETAS = [float(np.log(THETA / (l + 1) + 1.0)) for l in range(L)]

_cache = {}

LAST_PERF = {}


def _row_of_node(n):
    """Table row of global node id(s) n (vectorized)."""
    c = n // NOWN
    i = n - c * NOWN
    q = np.minimum(i // AGC, 3)
    tail = NLOC - 3 * AGC  # 256
    return np.where(q < 3, q * BANK + c * AGC + (i - q * AGC),
                    3 * BANK + c * tail + (i - 3 * AGC))


def _preprocess(edge_index):
    """All graph-structure preprocessing on host (numpy)."""
    e0 = edge_index[0].astype(np.int64)
    e1 = edge_index[1].astype(np.int64)
    # degrees INCLUDE self-loops (gcn_norm adds them), but self-loop edges are
    # folded on-chip and never gathered.
    deg = np.bincount(e1, minlength=N).astype(np.float64) + 1.0
    dinv = (1.0 / np.sqrt(deg)).astype(np.float32)

    src, dst = e0, e1
    owner = dst // NOWN
    ldst = dst - owner * NOWN
    t_arr = ldst // 128
    dl_arr = ldst % 128
    row = _row_of_node(src)
    b_arr = row // BANK
    bidx = (row % BANK).astype(np.int16)

    # group id and stable sort: (core, tile-group, bank, tile-in-group).
    # Bank-major within a tile group so each (tg, bank) is one contiguous
    # chunk range = one dma_gather call.
    NTG = (NT + TGS - 1) // TGS
    tg_arr = t_arr // TGS
    tl_arr = t_arr % TGS
    G = (((owner * NTG + tg_arr) * 4 + b_arr) * TGS + tl_arr).astype(np.int64)
    order = np.argsort(G, kind="stable")
    Gs = G[order]
    counts_g = np.bincount(Gs, minlength=NCORES * NTG * 4 * TGS)
    counts_g = counts_g.reshape(NCORES, NTG, 4, TGS)
    Cg = np.ceil(counts_g / 128).astype(np.int64).max(axis=0)  # [NTG, 4, TGS]

    # chunk offsets in (tg, bank, tile) order
    flat = Cg.reshape(-1)
    g_ch0 = np.zeros(len(flat) + 1, np.int64)
    np.cumsum(flat, out=g_ch0[1:])
    NCHUNK = int(g_ch0[-1])
    NSLOT = NCHUNK * 128
    ch_of = g_ch0[:-1].reshape(NTG, 4, TGS)   # first chunk of (tg, b, tl)
    slot0 = ch_of * 128

    # per-tile scattered chunk lists and per-(tg,bank) contiguous ranges
    tile_chunks = [
        [int(ch_of[t // TGS, b, t % TGS]) + k
         for b in range(4) for k in range(int(Cg[t // TGS, b, t % TGS]))]
        for t in range(NT)
    ]
    bank_rng = [
        [(int(ch_of[g, b, 0]),
          int(ch_of[g, b, 0]) + int(Cg[g, b, :].sum())) for b in range(4)]
        for g in range(NTG)
    ]

    # fill per-core flat arrays (vectorized scatter)
    gstart = np.zeros(NCORES * NTG * 4 * TGS + 1, np.int64)
    np.cumsum(counts_g.reshape(-1), out=gstart[1:])
    rank = np.arange(len(Gs)) - gstart[Gs]
    core_of = Gs // (NTG * 4 * TGS)
    tb = Gs % (NTG * 4 * TGS)
    dest = core_of * NSLOT + slot0.reshape(-1)[tb] + rank

    # padding gathers row 0 (indicator columns for padded slots are zero);
    # -1 trailing-trim is NOT used: it wedges the device, and with bank-major
    # batched gather calls padding is interior anyway.
    idx_flat = np.zeros(NCORES * NSLOT, np.int16)
    idx_flat[dest] = bidx[order]
    ind_flat = np.zeros((NCORES * NSLOT, 128), ml_dtypes.float8_e4m3fn)
    ind_flat[dest, dl_arr[order]] = 1.0

    idx_flat = idx_flat.reshape(NCORES, NSLOT)
    # idx device layout: [c, 128, NSLOT//16] (16-wrapped, replicated x8)
    idx_w = np.tile(
        idx_flat.reshape(NCORES, NSLOT // 16, 16).transpose(0, 2, 1), (1, 8, 1)
    ).copy()
    # ind device layout: [c, 128(slot%128), NCHUNK, 128(dl)]
    ind_w = np.ascontiguousarray(
        ind_flat.reshape(NCORES, NCHUNK, 128, 128).transpose(0, 2, 1, 3)
    ).reshape(NCORES, 128, NCHUNK * 128)

    dinv_loc = np.zeros((NCORES, NLOC), np.float32)
    dinv_loc[:, :NOWN] = dinv.reshape(NCORES, NOWN)
    dinv_col = np.ascontiguousarray(
        dinv_loc.reshape(NCORES, NT, 128).transpose(0, 2, 1))
    v09_col = (0.9 * dinv_col).astype(np.float32)

    return dict(tile_chunks=tile_chunks, bank_rng=bank_rng,
                NCHUNK=NCHUNK, NSLOT=NSLOT,
                idx_w=idx_w, ind_w=ind_w,
                dinv_col=dinv_col, v09_col=v09_col, counts=counts_g)


def _build_program(pre, n_layers=L, max_tg=None):
    nc = bacc.Bacc("TRN2", target_bir_lowering=False, debug=False,
                   num_devices=NCORES, num_swdge_queues=4)
    tile_chunks = pre["tile_chunks"]
    bank_rng = pre["bank_rng"]
    NCHUNK, NSLOT = pre["NCHUNK"], pre["NSLOT"]

    # ---- external inputs ----
    xT_in = nc.dram_tensor("xT", [128, KIN // 128, NLOC], F32, kind="ExternalInput")
    win_in = nc.dram_tensor("win", [128, KIN // 128, HID], F32, kind="ExternalInput")
    bin_in = nc.dram_tensor("bin", [128, 1], F32, kind="ExternalInput")
    wc_in = nc.dram_tensor("wc", [128, L, HID], F32, kind="ExternalInput")
    wout_in = nc.dram_tensor("wout", [128, OUT_C], F32, kind="ExternalInput")
    bout_in = nc.dram_tensor("bout", [128, OUT_C], F32, kind="ExternalInput")
    identf_in = nc.dram_tensor("identf", [128, 128], F32, kind="ExternalInput")
    identb_in = nc.dram_tensor("identb", [128, 128], BF16, kind="ExternalInput")
    dinv_in = nc.dram_tensor("dinvc", [128, NT], F32, kind="ExternalInput")
    v09_in = nc.dram_tensor("v09c", [128, NT], F32, kind="ExternalInput")
    idx_in = nc.dram_tensor("idx", [128, NSLOT // 16], I16, kind="ExternalInput")
    ind_in = nc.dram_tensor("ind", [128, NSLOT], FP8, kind="ExternalInput")
    out_ext = nc.dram_tensor("out", [NOWN, OUT_C], F32, kind="ExternalOutput")

    rg = [list(range(NCORES))]

    tg_list = [list(range(g, min(g + TGS, NT))) for g in range(0, NT, TGS)]
    if max_tg is not None:
        tg_list = tg_list[:max_tg]
    tg_c0 = [bank_rng[gi][0][0] for gi in range(len(tg_list))]
    tg_c1 = [bank_rng[gi][3][1] for gi in range(len(tg_list))]
    max_nch = max(c1 - c0 for c0, c1 in zip(tg_c0, tg_c1))

    with tile.TileContext(nc, num_cores=NCORES) as tc:
        with (
            tc.tile_pool(name="const", bufs=1) as cpool,
            tc.tile_pool(name="dram", bufs=1, space="DRAM") as dram,
            tc.tile_pool(name="work", bufs=1) as wp,
            tc.tile_pool(name="psum", bufs=1, space="PSUM") as pp,
        ):
            # ---- resident constants ----
            win_sb = cpool.tile([128, KIN // 128, HID], F32)
            bin_sb = cpool.tile([128, 1], F32)
            wc_sb = cpool.tile([128, L, HID], F32)
            wout_sb = cpool.tile([128, OUT_C], F32)
            bout_sb = cpool.tile([128, OUT_C], F32)
            identf_sb = cpool.tile([128, 128], F32)
            identb_sb = cpool.tile([128, 128], BF16)
            dinv_sb = cpool.tile([128, NT], F32)
            v09_sb = cpool.tile([128, NT], F32)
            h0s = cpool.tile([128, NT, HID], BF16)     # 0.1*h0, rows-major
            rows_buf = cpool.tile([128, NT, HID], BF16)  # dinv*h rows, resident
            for sb_t, ext in ((win_sb, win_in), (bin_sb, bin_in), (wc_sb, wc_in),
                              (wout_sb, wout_in), (bout_sb, bout_in),
                              (identf_sb, identf_in), (identb_sb, identb_in),
                              (dinv_sb, dinv_in), (v09_sb, v09_in)):
                nc.sync.dma_start(sb_t[:], ext[:])

            # gather destination buffers: explicit triple buffer, zeroed once
            # (padded slots gather row 0; indicator columns for them are zero)
            gbufs = [cpool.tile([128, max_nch, HID], BF16, name=f"gb{i}")
                     for i in range(3)]
            for g in gbufs:
                nc.vector.memset(g[:], 0.0)

            # one Shared table tensor per (layer, bank)
            NBROWS = [BANK, BANK, BANK, TROWS - 3 * BANK]
            tables = [
                [dram.tile([NBROWS[b], HID], BF16, addr_space="Shared",
                           name=f"table{r}_{b}") for b in range(4)]
                for r in range(L)
            ]
            shards = [
                dram.tile([NLOC, HID], BF16, name=f"shard{i}") for i in range(2)
            ]

            AG_BOUNDS = [(0, AGC), (AGC, 2 * AGC), (2 * AGC, 3 * AGC),
                         (3 * AGC, NLOC)]
            AG_LAG = 10   # tiles of slack before firing, so the Pool queue
                          # does not stall waiting for the shard writes

            def ag_round(shard, tb, r):
                """AllGather round r: shard rows -> table bank r.

                Fired mid-layer (inputs long written) so the transfer overlaps
                the remainder of the layer instead of serializing at the
                boundary behind the whole layer's dma_gathers."""
                r0, r1 = AG_BOUNDS[r]
                nc.gpsimd.collective_compute(
                    "AllGather", mybir.AluOpType.bypass, replica_groups=rg,
                    ins=[shard[r0:r1, :].opt()],
                    outs=[tb[r][:, :].opt()],
                )

            def fire_due_ags(shard, tb, done_tile, state, lag=AG_LAG):
                while state["next"] < 4 and \
                        done_tile + 1 >= AG_BOUNDS[state["next"]][1] // 128 + lag:
                    ag_round(shard, tb, state["next"])
                    state["next"] += 1

            def fire_rest_ags(shard, tb, state):
                while state["next"] < 4:
                    ag_round(shard, tb, state["next"])
                    state["next"] += 1

            # ================= input layer: h0 = relu(x @ W_in + b_in) ======
            shard0 = shards[0]
            ag_state = {"next": 0}
            for g0 in range(0, NLOC, 512):
                w = min(512, NLOC - g0)
                psin = pp.tile([128, 512], F32, tag="pi", bufs=1)
                for k in range(KIN // 128):
                    xt = wp.tile([128, 512], F32, tag="xt", bufs=3)
                    nc.sync.dma_start(xt[:, :w], xT_in[:, k, g0:g0 + w])
                    nc.tensor.matmul(psin[:, :w], win_sb[:, k, :], xt[:, :w],
                                     start=(k == 0), stop=(k == KIN // 128 - 1))
                h0T = wp.tile([128, 512], F32, tag="h0T", bufs=2)
                nc.scalar.activation(h0T[:, :w], psin[:, :w],
                                     mybir.ActivationFunctionType.Relu,
                                     bias=bin_sb[:, 0:1])
                for j in range(w // 128):
                    t = g0 // 128 + j
                    tp = pp.tile([128, 128], F32, tag="zt", bufs=2)
                    nc.tensor.transpose(tp[:], h0T[:, j * 128:(j + 1) * 128],
                                        identf_sb[:])
                    nc.vector.tensor_scalar(h0s[:, t, :], tp[:], ALPHA, None,
                                            mybir.AluOpType.mult)
                    nc.scalar.activation(rows_buf[:, t, :], tp[:],
                                         mybir.ActivationFunctionType.Copy,
                                         scale=dinv_sb[:, t:t + 1])
                    nc.scalar.dma_start(shard0[t * 128:(t + 1) * 128, :],
                                        rows_buf[:, t, :])
                    fire_due_ags(shard0, tables[0], t, ag_state, lag=2)
            fire_rest_ags(shard0, tables[0], ag_state)

            # ========================= L layers =============================
            for l in range(n_layers):
                t_in = tables[l]
                shard = shards[(l + 1) % 2]
                last = l == L - 1
                ag_state = {"next": 0}
                skips = os.environ.get("GCN_SKIP", "").split(",")
                for gi, tiles in enumerate(tg_list):
                    c0 = tg_c0[gi]
                    c1 = tg_c1[gi]
                    nch = c1 - c0
                    idx_t = wp.tile([128, nch * 8], I16, tag="idxs", bufs=3)
                    nc.gpsimd.dma_start(idx_t[:], idx_in[:, c0 * 8:c1 * 8])
                    ind_t = wp.tile([128, nch * 128], FP8, tag="ind", bufs=4)
                    # split the indicator stream across two queues so its DMA
                    # descriptors spread over more engines (it was the
                    # critical serializer when loaded via one queue)
                    half = (nch // 2) * 128
                    nc.sync.dma_start(ind_t[:, :half],
                                      ind_in[:, c0 * 128:c0 * 128 + half])
                    nc.scalar.dma_start(ind_t[:, half:nch * 128],
                                        ind_in[:, c0 * 128 + half:c1 * 128])
                    gbuf = gbufs[gi % 3]
                    # one gather per (tile-group, bank): bank-major slot layout
                    # makes each bank's chunks contiguous across the group
                    for b in range(4):
                        if "gather" in skips:
                            continue
                        b0, b1 = bank_rng[gi][b]
                        cnt = b1 - b0
                        if cnt == 0:
                            continue
                        rel = b0 - c0
                        nc.gpsimd.dma_gather(
                            gbuf[:, rel:rel + cnt, :],
                            t_in[b][:, :],
                            idx_t[:, rel * 8:(rel + cnt) * 8],
                            cnt * 128, cnt * 128, HID,
                            single_packet=False,
                            queue_num=(gi + b) % 4,
                        )

                    ps_g = pp.tile([128, 512], F32, tag="ps", bufs=2)
                    for ti, t in enumerate(tiles):
                        chunks = [p - c0 for p in tile_chunks[t]]
                        if "mm" in skips:
                            chunks = []
                        ps = ps_g[:, ti * 128:(ti + 1) * 128]
                        for k, p in enumerate(chunks):
                            nc.tensor.matmul(ps, ind_t[:, p * 128:(p + 1) * 128],
                                             gbuf[:, p, :],
                                             start=(k == 0),
                                             stop=(k == len(chunks) - 1))
                    for ti, t in enumerate(tiles):
                        ps = ps_g[:, ti * 128:(ti + 1) * 128]
                        # z = v09*(S + r_prev) + 0.1*h0   (self-loop folded)
                        t1 = wp.tile([128, 128], F32, tag="t1", bufs=2)
                        nc.vector.tensor_tensor(t1[:], ps, rows_buf[:, t, :],
                                                mybir.AluOpType.add)
                        z = wp.tile([128, 128], F32, tag="z", bufs=2)
                        nc.vector.scalar_tensor_tensor(
                            z[:], t1[:], v09_sb[:, t:t + 1], h0s[:, t, :],
                            mybir.AluOpType.mult, mybir.AluOpType.add)
                        # zT for the transform matmul
                        ztp = pp.tile([128, 128], F32, tag="zt", bufs=2)
                        nc.tensor.transpose(ztp[:], z[:], identf_sb[:])
                        zt = wp.tile([128, 128], F32, tag="ztc", bufs=2)
                        nc.scalar.activation(zt[:], ztp[:],
                                             mybir.ActivationFunctionType.Copy)
                        psh = pp.tile([128, 128], F32, tag="ph", bufs=2)
                        nc.tensor.matmul(psh[:], zt[:], wc_sb[:, l, :],
                                         start=True, stop=True)
                        pre = wp.tile([128, 128], F32, tag="pre", bufs=2)
                        nc.vector.scalar_tensor_tensor(
                            pre[:], z[:], 1.0 - BETAS[l], psh[:],
                            mybir.AluOpType.mult, mybir.AluOpType.add)
                        if not last:
                            nc.scalar.activation(
                                rows_buf[:, t, :], pre[:],
                                mybir.ActivationFunctionType.Relu,
                                scale=dinv_sb[:, t:t + 1])
                            nc.scalar.dma_start(
                                shard[t * 128:(t + 1) * 128, :],
                                rows_buf[:, t, :])
                            fire_due_ags(shard, tables[l + 1], t, ag_state)
                        else:
                            h = wp.tile([128, 128], F32, tag="h", bufs=2)
                            nc.scalar.activation(
                                h[:], pre[:],
                                mybir.ActivationFunctionType.Relu)
                            htp = pp.tile([128, 128], F32, tag="zt", bufs=2)
                            nc.tensor.transpose(htp[:], h[:], identf_sb[:])
                            ht = wp.tile([128, 128], F32, tag="ztc", bufs=2)
                            nc.scalar.activation(
                                ht[:], htp[:],
                                mybir.ActivationFunctionType.Copy)
                            pso = pp.tile([128, OUT_C], F32, tag="po", bufs=1)
                            nc.tensor.matmul(pso[:], ht[:], wout_sb[:],
                                             start=True, stop=True)
                            ob = wp.tile([128, OUT_C], F32, tag="ob", bufs=3)
                            nc.vector.tensor_tensor(ob[:], pso[:], bout_sb[:],
                                                    mybir.AluOpType.add)
                            r0 = t * 128
                            r1 = min(r0 + 128, NOWN)
                            if r1 > r0:
                                nc.scalar.dma_start(out_ext[r0:r1, :],
                                                    ob[:r1 - r0, :])
                    # end tiles
                if not last:
                    fire_rest_ags(shard, tables[l + 1], ag_state)

    nc.compile()
    return nc


def _host_inputs(inputs, pre):
    x = np.asarray(inputs["x"], np.float32)
    W_in = np.asarray(inputs["W_in"], np.float32)
    b_in = np.asarray(inputs["b_in"], np.float32)
    W_conv = np.asarray(inputs["W_conv"], np.float32)
    W_out = np.asarray(inputs["W_out"], np.float32)
    b_out = np.asarray(inputs["b_out"], np.float32)
    betas = np.array(BETAS, np.float32)

    win_w = np.zeros((128, KIN // 128, HID), np.float32)
    for k in range(KIN // 128):
        rows = W_in[k * 128:min((k + 1) * 128, IN_C)]
        win_w[:rows.shape[0], k, :] = rows
    wc_w = (W_conv * betas[:, None, None]).transpose(1, 0, 2).copy()  # [128,L,128]
    identf_w = np.eye(128, dtype=np.float32)
    identb_w = np.eye(128, dtype=ml_dtypes.bfloat16)
    bout_w = np.tile(b_out[None, :], (128, 1)).astype(np.float32)
    bin_w = b_in.reshape(128, 1).astype(np.float32)

    xT_w = np.zeros((NCORES, 128, KIN // 128, NLOC), np.float32)
    xr = x.reshape(NCORES, NOWN, IN_C)
    for k in range(KIN // 128):
        c0, c1 = k * 128, min((k + 1) * 128, IN_C)
        xT_w[:, :c1 - c0, k, :NOWN] = xr[:, :, c0:c1].transpose(0, 2, 1)

    maps = []
    for c in range(NCORES):
        maps.append({
            "xT": xT_w[c], "win": win_w, "bin": bin_w, "wc": wc_w,
            "wout": W_out, "bout": bout_w,
            "identf": identf_w, "identb": identb_w,
            "dinvc": pre["dinv_col"][c], "v09c": pre["v09_col"][c],
            "idx": pre["idx_w"][c], "ind": pre["ind_w"][c],
        })
    return maps


def kernel(**inputs):
    edge_index = np.asarray(inputs["edge_index"])
    key = hash(edge_index.tobytes())
    if key not in _cache:
        pre = _preprocess(edge_index)
        n_layers = int(os.environ.get("GCN_NL", L))
        max_tg = os.environ.get("GCN_MAXTG")
        nc = _build_program(pre, n_layers,
                            int(max_tg) if max_tg else None)
        _cache.clear()
        _cache[key] = (pre, nc)
    pre, nc = _cache[key]

    in_maps = _host_inputs(inputs, pre)
    trace = bool(os.environ.get("GCN_TRACE"))
    res = run_bass_kernel_spmd(nc, in_maps, core_ids=list(range(NCORES)),
                               trace=trace)
    LAST_PERF["exec_time_ns"] = res.exec_time_ns
    LAST_PERF["mean_exec_time_ns"] = res.mean_exec_time_ns
    LAST_PERF["trace"] = (res.instructions_and_trace or (None, None))[1]
    out = np.concatenate([res.results[c]["out"] for c in range(NCORES)], axis=0)
    return out.astype(np.float32)


# revision 35
# speedup vs baseline: 1.0834x; 1.0430x over previous
"""GCN2 (GCNII) message-passing kernel for 8 Trainium2 NeuronCores.

Strategy (1D node partition, hint-compliant):
  - Nodes sharded 8 ways by id; each core owns 12500 nodes (padded to 12544).
  - Layer weights replicated; per-layer halo exchange realized as 4 chunked
    AllGathers of each core's row shard into a replicated bf16 DRAM table.
  - Symmetric normalization folded into the data path:
      table rows store r = dinv * h  (bf16); per-dst scale v09 = 0.9*dinv[dst]
      applied AFTER aggregation (per-partition scalar in rows-major layout).
  - Self-loops never gathered: their contribution is v09[d] * r[d] using the
    resident local rows, fused into the z computation.
  - Aggregation (segment_sum over dst-sorted edges): dma_gather of 256B bf16
    rows by src, then per-128-edge-chunk one-hot matmuls on the TensorEngine
    accumulating into PSUM (S[dst,feat] += onehot^T @ G).
    One-hot indicators are PURE 0/1, layer-invariant, precomputed on host and
    streamed from DRAM (no per-chunk VectorEngine builds).
  - Edges grouped by (dst tile of 128, src bank of 32768 rows) because
    dma_gather indices are int16; per-core padding is trailing -1 indices per
    (tile,bank) gather call, which the gather ucode drops (no descriptors, no
    HBM fetch for padding).
  - Transform matmuls (z @ W) and input/output layers kept in fp32 (cheap,
    preserves accuracy); only the gather/aggregate path is bf16.
"""

import math
import os
import sys

import numpy as np

for _p in ("/opt/trn_rl_repo",):
    if _p not in sys.path and os.path.isdir(_p):
        sys.path.insert(0, _p)

import ml_dtypes

import concourse.bacc as bacc
import concourse.mybir as mybir
import concourse.tile as tile
from concourse.bass_utils import run_bass_kernel_spmd

# ---------------- problem constants (hardcoded per contract) ----------------
N = 100_000
E = 1_600_000
IN_C = 500
HID = 128
OUT_C = 64
L = 8
ALPHA = 0.1
THETA = 0.5

NCORES = 8
NOWN = N // NCORES          # 12500 real nodes per core
NLOC = 12544                # padded to 98 * 128
NT = NLOC // 128            # 98 dst tiles per core
TGS = 4                     # dst tiles per load group
KIN = 512                   # padded input channels
BANK = 32768                # int16-addressable rows per gather bank
AGC = 4096                  # shard rows per chunked AllGather (AG 0..2)
TROWS = 3 * BANK + NCORES * (NLOC - 3 * AGC)   # 100352 table rows

F32 = mybir.dt.float32
BF16 = mybir.dt.bfloat16
FP8 = mybir.dt.float8e4
I16 = mybir.dt.int16
BETAS = [float(np.log(THETA / (l + 1) + 1.0)) for l in range(L)]

_cache = {}

LAST_PERF = {}


def _row_of_node(n):
    """Table row of global node id(s) n (vectorized)."""
    c = n // NOWN
    i = n - c * NOWN
    q = np.minimum(i // AGC, 3)
    tail = NLOC - 3 * AGC  # 256
    return np.where(q < 3, q * BANK + c * AGC + (i - q * AGC),
                    3 * BANK + c * tail + (i - 3 * AGC))


def _preprocess(edge_index):
    """All graph-structure preprocessing on host (numpy)."""
    e0 = edge_index[0].astype(np.int64)
    e1 = edge_index[1].astype(np.int64)
    # degrees INCLUDE self-loops (gcn_norm adds them), but self-loop edges are
    # folded on-chip and never gathered.
    deg = np.bincount(e1, minlength=N).astype(np.float64) + 1.0
    dinv = (1.0 / np.sqrt(deg)).astype(np.float32)

    src, dst = e0, e1
    owner = dst // NOWN
    ldst = dst - owner * NOWN
    t_arr = ldst // 128
    dl_arr = ldst % 128
    row = _row_of_node(src)
    b_arr = row // BANK
    bidx = (row % BANK).astype(np.int16)

    # group id and stable sort: (core, tile-group, bank, tile-in-group).
    # Bank-major within a tile group so each (tg, bank) is one contiguous
    # chunk range = one dma_gather call.
    NTG = (NT + TGS - 1) // TGS
    tg_arr = t_arr // TGS
    tl_arr = t_arr % TGS
    G = (((owner * NTG + tg_arr) * 4 + b_arr) * TGS + tl_arr).astype(np.int64)
    order = np.argsort(G, kind="stable")
    Gs = G[order]
    counts_g = np.bincount(Gs, minlength=NCORES * NTG * 4 * TGS)
    counts_g = counts_g.reshape(NCORES, NTG, 4, TGS)
    Cg = np.ceil(counts_g / 128).astype(np.int64).max(axis=0)  # [NTG, 4, TGS]

    # chunk offsets in (tg, bank, tile) order
    flat = Cg.reshape(-1)
    g_ch0 = np.zeros(len(flat) + 1, np.int64)
    np.cumsum(flat, out=g_ch0[1:])
    NCHUNK = int(g_ch0[-1])
    NSLOT = NCHUNK * 128
    ch_of = g_ch0[:-1].reshape(NTG, 4, TGS)   # first chunk of (tg, b, tl)
    slot0 = ch_of * 128

    # per-tile scattered chunk lists and per-(tg,bank) contiguous ranges
    tile_chunks = [
        [int(ch_of[t // TGS, b, t % TGS]) + k
         for b in range(4) for k in range(int(Cg[t // TGS, b, t % TGS]))]
        for t in range(NT)
    ]
    bank_rng = [
        [(int(ch_of[g, b, 0]),
          int(ch_of[g, b, 0]) + int(Cg[g, b, :].sum())) for b in range(4)]
        for g in range(NTG)
    ]

    # fill per-core flat arrays (vectorized scatter)
    gstart = np.zeros(NCORES * NTG * 4 * TGS + 1, np.int64)
    np.cumsum(counts_g.reshape(-1), out=gstart[1:])
    rank = np.arange(len(Gs)) - gstart[Gs]
    core_of = Gs // (NTG * 4 * TGS)
    tb = Gs % (NTG * 4 * TGS)
    dest = core_of * NSLOT + slot0.reshape(-1)[tb] + rank

    # padding gathers row 0 (indicator columns for padded slots are zero);
    # -1 trailing-trim is NOT used: it wedges the device, and with bank-major
    # batched gather calls padding is interior anyway.
    idx_flat = np.zeros(NCORES * NSLOT, np.int16)
    idx_flat[dest] = bidx[order]
    ind_flat = np.zeros((NCORES * NSLOT, 128), ml_dtypes.float8_e4m3fn)
    ind_flat[dest, dl_arr[order]] = 1.0

    idx_flat = idx_flat.reshape(NCORES, NSLOT)
    # idx device layout: [c, 128, NSLOT//16] (16-wrapped, replicated x8)
    idx_w = np.tile(
        idx_flat.reshape(NCORES, NSLOT // 16, 16).transpose(0, 2, 1), (1, 8, 1)
    ).copy()
    # ind device layout: [c, 128(slot%128), NCHUNK, 128(dl)]
    ind_w = np.ascontiguousarray(
        ind_flat.reshape(NCORES, NCHUNK, 128, 128).transpose(0, 2, 1, 3)
    ).reshape(NCORES, 128, NCHUNK * 128)

    dinv_loc = np.zeros((NCORES, NLOC), np.float32)
    dinv_loc[:, :NOWN] = dinv.reshape(NCORES, NOWN)
    dinv_col = np.ascontiguousarray(
        dinv_loc.reshape(NCORES, NT, 128).transpose(0, 2, 1))
    v09_col = (0.9 * dinv_col).astype(np.float32)

    return dict(tile_chunks=tile_chunks, bank_rng=bank_rng,
                Cg=Cg, ch_of=ch_of,
                NCHUNK=NCHUNK, NSLOT=NSLOT,
                idx_w=idx_w, ind_w=ind_w,
                dinv_col=dinv_col, v09_col=v09_col, counts=counts_g)


def _build_program(pre, n_layers=L, max_tg=None):
    nc = bacc.Bacc("TRN2", target_bir_lowering=False, debug=False,
                   num_devices=NCORES, num_swdge_queues=4)
    tile_chunks = pre["tile_chunks"]
    bank_rng = pre["bank_rng"]
    Cg, ch_of = pre["Cg"], pre["ch_of"]
    NCHUNK, NSLOT = pre["NCHUNK"], pre["NSLOT"]

    # ---- external inputs ----
    xT_in = nc.dram_tensor("xT", [128, KIN // 128, NLOC], F32, kind="ExternalInput")
    win_in = nc.dram_tensor("win", [128, KIN // 128, HID], F32, kind="ExternalInput")
    bin_in = nc.dram_tensor("bin", [128, 1], F32, kind="ExternalInput")
    wc_in = nc.dram_tensor("wc", [128, L, HID], F32, kind="ExternalInput")
    wout_in = nc.dram_tensor("wout", [128, OUT_C], F32, kind="ExternalInput")
    bout_in = nc.dram_tensor("bout", [128, OUT_C], F32, kind="ExternalInput")
    identf_in = nc.dram_tensor("identf", [128, 128], F32, kind="ExternalInput")
    identb_in = nc.dram_tensor("identb", [128, 128], BF16, kind="ExternalInput")
    dinv_in = nc.dram_tensor("dinvc", [128, NT], F32, kind="ExternalInput")
    v09_in = nc.dram_tensor("v09c", [128, NT], F32, kind="ExternalInput")
    idx_in = nc.dram_tensor("idx", [128, NSLOT // 16], I16, kind="ExternalInput")
    ind_in = nc.dram_tensor("ind", [128, NSLOT], FP8, kind="ExternalInput")
    out_ext = nc.dram_tensor("out", [NOWN, OUT_C], F32, kind="ExternalOutput")

    rg = [list(range(NCORES))]

    tg_list = [list(range(g, min(g + TGS, NT))) for g in range(0, NT, TGS)]
    if max_tg is not None:
        tg_list = tg_list[:max_tg]
    tg_c0 = [bank_rng[gi][0][0] for gi in range(len(tg_list))]
    tg_c1 = [bank_rng[gi][3][1] for gi in range(len(tg_list))]
    max_nch = max(c1 - c0 for c0, c1 in zip(tg_c0, tg_c1))

    with tile.TileContext(nc, num_cores=NCORES) as tc:
        with (
            tc.tile_pool(name="const", bufs=1) as cpool,
            tc.tile_pool(name="dram", bufs=1, space="DRAM") as dram,
            tc.tile_pool(name="work", bufs=1) as wp,
            tc.tile_pool(name="psum", bufs=1, space="PSUM") as pp,
        ):
            # ---- resident constants ----
            win_sb = cpool.tile([128, KIN // 128, HID], F32)
            bin_sb = cpool.tile([128, 1], F32)
            wc_sb = cpool.tile([128, L, HID], F32)
            wout_sb = cpool.tile([128, OUT_C], F32)
            bout_sb = cpool.tile([128, OUT_C], F32)
            identf_sb = cpool.tile([128, 128], F32)
            identb_sb = cpool.tile([128, 128], BF16)
            dinv_sb = cpool.tile([128, NT], F32)
            v09_sb = cpool.tile([128, NT], F32)
            h0s = cpool.tile([128, NT, HID], BF16)     # 0.1*h0, rows-major
            rows_buf = cpool.tile([128, NT, HID], BF16)  # dinv*h rows, resident
            for sb_t, ext in ((win_sb, win_in), (bin_sb, bin_in), (wc_sb, wc_in),
                              (wout_sb, wout_in), (bout_sb, bout_in),
                              (identf_sb, identf_in), (identb_sb, identb_in),
                              (dinv_sb, dinv_in), (v09_sb, v09_in)):
                nc.sync.dma_start(sb_t[:], ext[:])

            # gather destination buffers: explicit triple buffer, zeroed once
            # (padded slots gather row 0; indicator columns for them are zero)
            gbufs = [cpool.tile([128, max_nch, HID], BF16, name=f"gb{i}")
                     for i in range(3)]
            for g in gbufs:
                nc.vector.memset(g[:], 0.0)

            # one Shared table tensor per (layer, bank)
            NBROWS = [BANK, BANK, BANK, TROWS - 3 * BANK]
            tables = [
                [dram.tile([NBROWS[b], HID], BF16, addr_space="Shared",
                           name=f"table{r}_{b}") for b in range(4)]
                for r in range(L)
            ]
            shards = [
                dram.tile([NLOC, HID], BF16, name=f"shard{i}") for i in range(2)
            ]

            AG_BOUNDS = [(0, AGC), (AGC, 2 * AGC), (2 * AGC, 3 * AGC),
                         (3 * AGC, NLOC)]
            AG_LAG = 10   # tiles of slack before firing, so the Pool queue
                          # does not stall waiting for the shard writes

            def ag_round(shard, tb, r):
                """AllGather round r: shard rows -> table bank r.

                Fired mid-layer (inputs long written) so the transfer overlaps
                the remainder of the layer instead of serializing at the
                boundary behind the whole layer's dma_gathers."""
                r0, r1 = AG_BOUNDS[r]
                nc.gpsimd.collective_compute(
                    "AllGather", mybir.AluOpType.bypass, replica_groups=rg,
                    ins=[shard[r0:r1, :].opt()],
                    outs=[tb[r][:, :].opt()],
                )

            def fire_due_ags(shard, tb, done_tile, state, lag=AG_LAG):
                while state["next"] < 4 and \
                        done_tile + 1 >= AG_BOUNDS[state["next"]][1] // 128 + lag:
                    ag_round(shard, tb, state["next"])
                    state["next"] += 1

            def fire_rest_ags(shard, tb, state):
                while state["next"] < 4:
                    ag_round(shard, tb, state["next"])
                    state["next"] += 1

            # ================= input layer: h0 = relu(x @ W_in + b_in) ======
            shard0 = shards[0]
            ag_state = {"next": 0}
            for g0 in range(0, NLOC, 512):
                w = min(512, NLOC - g0)
                psin = pp.tile([128, 512], F32, tag="pi", bufs=1)
                for k in range(KIN // 128):
                    xt = wp.tile([128, 512], F32, tag="xt", bufs=3)
                    nc.sync.dma_start(xt[:, :w], xT_in[:, k, g0:g0 + w])
                    nc.tensor.matmul(psin[:, :w], win_sb[:, k, :], xt[:, :w],
                                     start=(k == 0), stop=(k == KIN // 128 - 1))
                h0T = wp.tile([128, 512], F32, tag="h0T", bufs=2)
                nc.scalar.activation(h0T[:, :w], psin[:, :w],
                                     mybir.ActivationFunctionType.Relu,
                                     bias=bin_sb[:, 0:1])
                for j in range(w // 128):
                    t = g0 // 128 + j
                    tp = pp.tile([128, 128], F32, tag="zt", bufs=2)
                    nc.tensor.transpose(tp[:], h0T[:, j * 128:(j + 1) * 128],
                                        identf_sb[:])
                    nc.vector.tensor_scalar(h0s[:, t, :], tp[:], ALPHA, None,
                                            mybir.AluOpType.mult)
                    nc.scalar.activation(rows_buf[:, t, :], tp[:],
                                         mybir.ActivationFunctionType.Copy,
                                         scale=dinv_sb[:, t:t + 1])
                    nc.scalar.dma_start(shard0[t * 128:(t + 1) * 128, :],
                                        rows_buf[:, t, :])
                    fire_due_ags(shard0, tables[0], t, ag_state, lag=2)
            fire_rest_ags(shard0, tables[0], ag_state)

            # ========================= L layers =============================
            for l in range(n_layers):
                t_in = tables[l]
                shard = shards[(l + 1) % 2]
                last = l == L - 1
                ag_state = {"next": 0}
                skips = os.environ.get("GCN_SKIP", "").split(",")
                for gi, tiles in enumerate(tg_list):
                    c0 = tg_c0[gi]
                    c1 = tg_c1[gi]
                    nch = c1 - c0
                    idx_t = wp.tile([128, nch * 8], I16, tag="idxs", bufs=3)
                    nc.gpsimd.dma_start(idx_t[:], idx_in[:, c0 * 8:c1 * 8])
                    ind_t = wp.tile([128, nch * 128], FP8, tag="ind", bufs=4)
                    # split the indicator stream across two queues so its DMA
                    # descriptors spread over more engines (it was the
                    # critical serializer when loaded via one queue)
                    half = (nch // 2) * 128
                    nc.sync.dma_start(ind_t[:, :half],
                                      ind_in[:, c0 * 128:c0 * 128 + half])
                    nc.scalar.dma_start(ind_t[:, half:nch * 128],
                                        ind_in[:, c0 * 128 + half:c1 * 128])
                    gbuf = gbufs[gi % 3]
                    # one gather per (tile, bank): the SWDGE transfer stall is
                    # superlinear in call size, so small calls across the 4
                    # queues pipeline much better than (tile-group, bank) ones
                    for b in range(4):
                        if "gather" in skips:
                            continue
                        for ti, t in enumerate(tiles):
                            cnt = int(Cg[gi, b, ti])
                            if cnt == 0:
                                continue
                            rel = int(ch_of[gi, b, ti]) - c0
                            nc.gpsimd.dma_gather(
                                gbuf[:, rel:rel + cnt, :],
                                t_in[b][:, :],
                                idx_t[:, rel * 8:(rel + cnt) * 8],
                                cnt * 128, cnt * 128, HID,
                                single_packet=False,
                                queue_num=(t + b) % 4,
                            )

                    ps_g = pp.tile([128, 512], F32, tag="ps", bufs=2)
                    for ti, t in enumerate(tiles):
                        chunks = [p - c0 for p in tile_chunks[t]]
                        if "mm" in skips:
                            chunks = []
                        ps = ps_g[:, ti * 128:(ti + 1) * 128]
                        for k, p in enumerate(chunks):
                            nc.tensor.matmul(ps, ind_t[:, p * 128:(p + 1) * 128],
                                             gbuf[:, p, :],
                                             start=(k == 0),
                                             stop=(k == len(chunks) - 1))
                    for ti, t in enumerate(tiles):
                        ps = ps_g[:, ti * 128:(ti + 1) * 128]
                        # z = v09*(S + r_prev) + 0.1*h0   (self-loop folded)
                        t1 = wp.tile([128, 128], F32, tag="t1", bufs=2)
                        nc.vector.tensor_tensor(t1[:], ps, rows_buf[:, t, :],
                                                mybir.AluOpType.add)
                        z = wp.tile([128, 128], F32, tag="z", bufs=2)
                        nc.vector.scalar_tensor_tensor(
                            z[:], t1[:], v09_sb[:, t:t + 1], h0s[:, t, :],
                            mybir.AluOpType.mult, mybir.AluOpType.add)
                        # zT for the transform matmul
                        ztp = pp.tile([128, 128], F32, tag="zt", bufs=2)
                        nc.tensor.transpose(ztp[:], z[:], identf_sb[:])
                        zt = wp.tile([128, 128], F32, tag="ztc", bufs=2)
                        nc.scalar.activation(zt[:], ztp[:],
                                             mybir.ActivationFunctionType.Copy)
                        psh = pp.tile([128, 128], F32, tag="ph", bufs=2)
                        nc.tensor.matmul(psh[:], zt[:], wc_sb[:, l, :],
                                         start=True, stop=True)
                        pre = wp.tile([128, 128], F32, tag="pre", bufs=2)
                        nc.vector.scalar_tensor_tensor(
                            pre[:], z[:], 1.0 - BETAS[l], psh[:],
                            mybir.AluOpType.mult, mybir.AluOpType.add)
                        if not last:
                            nc.scalar.activation(
                                rows_buf[:, t, :], pre[:],
                                mybir.ActivationFunctionType.Relu,
                                scale=dinv_sb[:, t:t + 1])
                            nc.scalar.dma_start(
                                shard[t * 128:(t + 1) * 128, :],
                                rows_buf[:, t, :])
                            fire_due_ags(shard, tables[l + 1], t, ag_state)
                        else:
                            h = wp.tile([128, 128], F32, tag="h", bufs=2)
                            nc.scalar.activation(
                                h[:], pre[:],
                                mybir.ActivationFunctionType.Relu)
                            htp = pp.tile([128, 128], F32, tag="zt", bufs=2)
                            nc.tensor.transpose(htp[:], h[:], identf_sb[:])
                            ht = wp.tile([128, 128], F32, tag="ztc", bufs=2)
                            nc.scalar.activation(
                                ht[:], htp[:],
                                mybir.ActivationFunctionType.Copy)
                            pso = pp.tile([128, OUT_C], F32, tag="po", bufs=1)
                            nc.tensor.matmul(pso[:], ht[:], wout_sb[:],
                                             start=True, stop=True)
                            ob = wp.tile([128, OUT_C], F32, tag="ob", bufs=3)
                            nc.vector.tensor_tensor(ob[:], pso[:], bout_sb[:],
                                                    mybir.AluOpType.add)
                            r0 = t * 128
                            r1 = min(r0 + 128, NOWN)
                            if r1 > r0:
                                nc.scalar.dma_start(out_ext[r0:r1, :],
                                                    ob[:r1 - r0, :])
                    # end tiles
                if not last:
                    fire_rest_ags(shard, tables[l + 1], ag_state)

    nc.compile()
    return nc


def _host_inputs(inputs, pre):
    x = np.asarray(inputs["x"], np.float32)
    W_in = np.asarray(inputs["W_in"], np.float32)
    b_in = np.asarray(inputs["b_in"], np.float32)
    W_conv = np.asarray(inputs["W_conv"], np.float32)
    W_out = np.asarray(inputs["W_out"], np.float32)
    b_out = np.asarray(inputs["b_out"], np.float32)
    betas = np.array(BETAS, np.float32)

    win_w = np.zeros((128, KIN // 128, HID), np.float32)
    for k in range(KIN // 128):
        rows = W_in[k * 128:min((k + 1) * 128, IN_C)]
        win_w[:rows.shape[0], k, :] = rows
    wc_w = (W_conv * betas[:, None, None]).transpose(1, 0, 2).copy()  # [128,L,128]
    identf_w = np.eye(128, dtype=np.float32)
    identb_w = np.eye(128, dtype=ml_dtypes.bfloat16)
    bout_w = np.tile(b_out[None, :], (128, 1)).astype(np.float32)
    bin_w = b_in.reshape(128, 1).astype(np.float32)

    xT_w = np.zeros((NCORES, 128, KIN // 128, NLOC), np.float32)
    xr = x.reshape(NCORES, NOWN, IN_C)
    for k in range(KIN // 128):
        c0, c1 = k * 128, min((k + 1) * 128, IN_C)
        xT_w[:, :c1 - c0, k, :NOWN] = xr[:, :, c0:c1].transpose(0, 2, 1)

    maps = []
    for c in range(NCORES):
        maps.append({
            "xT": xT_w[c], "win": win_w, "bin": bin_w, "wc": wc_w,
            "wout": W_out, "bout": bout_w,
            "identf": identf_w, "identb": identb_w,
            "dinvc": pre["dinv_col"][c], "v09c": pre["v09_col"][c],
            "idx": pre["idx_w"][c], "ind": pre["ind_w"][c],
        })
    return maps


def kernel(**inputs):
    edge_index = np.asarray(inputs["edge_index"])
    key = hash(edge_index.tobytes())
    if key not in _cache:
        pre = _preprocess(edge_index)
        n_layers = int(os.environ.get("GCN_NL", L))
        max_tg = os.environ.get("GCN_MAXTG")
        nc = _build_program(pre, n_layers,
                            int(max_tg) if max_tg else None)
        _cache.clear()
        _cache[key] = (pre, nc)
    pre, nc = _cache[key]

    in_maps = _host_inputs(inputs, pre)
    trace = bool(os.environ.get("GCN_TRACE"))
    res = run_bass_kernel_spmd(nc, in_maps, core_ids=list(range(NCORES)),
                               trace=trace)
    LAST_PERF["exec_time_ns"] = res.exec_time_ns
    LAST_PERF["mean_exec_time_ns"] = res.mean_exec_time_ns
    LAST_PERF["trace"] = (res.instructions_and_trace or (None, None))[1]
    out = np.concatenate([res.results[c]["out"] for c in range(NCORES)], axis=0)
    return out.astype(np.float32)
